# revision 1
# baseline (speedup 1.0000x reference)
"""Trainium2 Bass kernel for the KGEncoder RGCN (nn_KGEncoder_14027363188782).

Math (per batch element b, L=5 layers):
    x0 = ent_emb                                             (E, D)
    per layer i:
      y_r   = x @ Wb_x[i,r] + 1 * c[i,r]^T    (E, NB)  where c[i,r] = rel_r @ Wb_rel[i,r]
      Z     = sum_r adj_r @ y_r               (E, NB)  == sup @ Wb[i]  (deg term folded via c)
      h     = relu(Z @ Ww[i] + bias[i])
      g     = sigmoid(h @ Wh[i] + bh[i])
      x     = x + g * (h - x)
    out_b = sum_e x[e] * m[e] / max(sum_e m[e], 1)

Sharding: core c handles b = c // 2 (pair-replicated, no collectives).
adj is shipped pre-transposed (j-major) in bf16 (exact for 0/1 values).
Big matmul: out Z.T (NB x E) = sum_{r,k} y'[kchunk]_r.T @ adjT_r[kchunk];
NRES relations stay resident in SBUF, the rest stream from HBM each layer.
"""

import numpy as np
import ml_dtypes

import concourse.bacc as bacc
import concourse.bass as bass
import concourse.mybir as mybir
import concourse.tile as tile
from concourse import bass_utils
from concourse.bass import MemorySpace

B, R, E, D, HID, L, NB = 4, 10, 1500, 100, 100, 5, 3
EP = 1536           # entity (j) dim padded to 12*128
CH = EP // 128      # 12 k-chunks
FP8 = True          # fp8 adj (exact for 0/1) -> all relations SBUF-resident
DR = True           # DoubleRow fp8 matmul: 256-deep contraction, 2 elem/lane/cyc
C2 = 6              # 256-row contraction chunks (DoubleRow)
E2 = 1504           # i dim padded to 16-aligned for DoubleRow strides
YQ = 32             # y_all per-chunk col stride (16-aligned)
NRES = 10 if FP8 else 4   # relations resident in SBUF
SG = 3              # k-chunks per streamed stage tile
NW = 500            # psum free-dim chunk (3 per row of E)
RNB = R * NB        # 30
f32 = mybir.dt.float32
bf16 = mybir.dt.bfloat16
ADT = mybir.dt.float8e4 if FP8 else mybir.dt.bfloat16
ADT_NP = ml_dtypes.float8_e4m3fn if FP8 else ml_dtypes.bfloat16
AF = mybir.ActivationFunctionType
AX = mybir.AxisListType

_NC_CACHE = {}


def _build_nc():
    nc = bacc.Bacc("TRN2", target_bir_lowering=False, debug=False)

    if DR:
        adjT = nc.dram_tensor(
            "adjT", [R, C2, 128, 2, E2], ADT, kind="ExternalInput"
        ).ap()
    else:
        adjT = nc.dram_tensor("adjT", [R, EP, E], ADT, kind="ExternalInput").ap()
    xT0 = nc.dram_tensor("xT0", [D, E], f32, kind="ExternalInput").ap()
    maskrep = nc.dram_tensor("maskrep", [HID, E], f32, kind="ExternalInput").ap()
    relT = nc.dram_tensor("relT", [D, R], f32, kind="ExternalInput").ap()
    wbxD = nc.dram_tensor("wbx", [L, D, RNB], f32, kind="ExternalInput").ap()
    wbrD = nc.dram_tensor("wbr", [L, D, RNB], f32, kind="ExternalInput").ap()
    wwD = nc.dram_tensor("ww", [L, NB, HID], f32, kind="ExternalInput").ap()
    whD = nc.dram_tensor("wh", [L, HID, HID], f32, kind="ExternalInput").ap()
    biasD = nc.dram_tensor("biasL", [L, HID], f32, kind="ExternalInput").ap()
    bhD = nc.dram_tensor("bhL", [L, HID], f32, kind="ExternalInput").ap()
    graphD = nc.dram_tensor("graph", [HID, 1], f32, kind="ExternalOutput").ap()

    with tile.TileContext(nc) as tc:
        with (
            tc.tile_pool(name="singles", bufs=1) as singles,
            tc.tile_pool(name="resp", bufs=1) as resp,
            tc.tile_pool(name="stagep", bufs=4) as stagep,
            tc.tile_pool(name="ypool", bufs=2) as ypool,
            tc.tile_pool(name="workp", bufs=2) as workp,
            tc.tile_pool(name="psY", bufs=1, space=MemorySpace.PSUM) as psY,
            tc.tile_pool(name="psC", bufs=1, space=MemorySpace.PSUM) as psC,
            tc.tile_pool(name="psB", bufs=1, space=MemorySpace.PSUM) as psB,
        ):
            # ---- persistent state ----
            xT = singles.tile([D, EP], f32, tag="xT", name="xT")
            nc.sync.dma_start(out=xT[:, 0:E], in_=xT0)
            nc.vector.memset(xT[:, E:EP], 0.0)

            ones = singles.tile([1, 128], f32, tag="ones", name="ones")
            nc.vector.memset(ones[:, :], 1.0)

            mask_sb = singles.tile([HID, E], f32, tag="mask", name="mask_sb")
            nc.sync.dma_start(out=mask_sb[:, :], in_=maskrep)

            relT_sb = singles.tile([D, R], f32, tag="relT", name="relT_sb")
            nc.sync.dma_start(out=relT_sb[:, :], in_=relT)

            wbx_sb, wbr_sb, ww_sb, wh_sb, bias_sb, bh_sb = [], [], [], [], [], []
            for i in range(L):
                wx = singles.tile([D, RNB], f32, tag=f"wbx{i}", name=f"wbx{i}")
                nc.sync.dma_start(out=wx[:, :], in_=wbxD[i])
                wbx_sb.append(wx)
                wr = singles.tile([D, RNB], f32, tag=f"wbr{i}", name=f"wbr{i}")
                nc.sync.dma_start(out=wr[:, :], in_=wbrD[i])
                wbr_sb.append(wr)
                wwt = singles.tile([NB, HID], f32, tag=f"ww{i}", name=f"ww{i}")
                nc.sync.dma_start(out=wwt[:, :], in_=wwD[i])
                ww_sb.append(wwt)
                wht = singles.tile([HID, HID], f32, tag=f"wh{i}", name=f"wh{i}")
                nc.sync.dma_start(out=wht[:, :], in_=whD[i])
                wh_sb.append(wht)
                bt = singles.tile([HID, 1], f32, tag=f"bias{i}", name=f"bias{i}")
                nc.sync.dma_start(out=bt[:, :], in_=biasD[i].unsqueeze(1))
                bias_sb.append(bt)
                bht = singles.tile([HID, 1], f32, tag=f"bh{i}", name=f"bh{i}")
                nc.sync.dma_start(out=bht[:, :], in_=bhD[i].unsqueeze(1))
                bh_sb.append(bht)

            # resident adjT relations: tile (128, CH*E), chunk k at cols [k*E, (k+1)*E)
            res_tiles = []
            for r in range(NRES):
                if DR:
                    rt = resp.tile([128, C2 * 2 * E2], ADT,
                                   tag=f"res{r}", name=f"res{r}")
                    nc.sync.dma_start(
                        out=rt[:, :].rearrange("p (c t i) -> p c t i", c=C2, t=2),
                        in_=adjT[r].rearrange("c p t i -> p c t i"),
                    )
                else:
                    rt = resp.tile([128, CH * E], ADT, tag=f"res{r}", name=f"res{r}")
                    nc.sync.dma_start(
                        out=rt[:, :].rearrange("p (k i) -> p k i", k=CH),
                        in_=adjT[r].rearrange("(k p) i -> p k i", p=128),
                    )
                res_tiles.append(rt)

            # ---- layers ----
            for i in range(L):
                # c[i, r, :] = rel_r @ Wb_rel[i, r]   -> psum row 0, cols 3r..3r+3
                psc = psC.tile([1, RNB], f32, tag="c", name=f"psc{i}")
                for r in range(R):
                    nc.tensor.matmul(
                        psc[:, 3 * r : 3 * r + 3],
                        relT_sb[:, r : r + 1],
                        wbr_sb[i][:, 3 * r : 3 * r + 3],
                        start=True, stop=True,
                    )
                c_sb = workp.tile([1, RNB], f32, tag="c_sb", name=f"c_sb{i}", bufs=2)
                nc.scalar.copy(out=c_sb[:, :], in_=psc[:, :])

                # y'[kchunk] = x[kchunk] @ Wbx[i]  + 1 (x) c   -> bf16 (128, RNB) per chunk
                YS = YQ if DR else RNB
                y_all = ypool.tile([128, CH * YS], ADT, tag="y_all", name=f"y_all{i}")
                for k in range(CH):
                    psy = psY.tile([128, RNB], f32, tag="y", name=f"psy{i}_{k}")
                    nc.tensor.matmul(
                        psy[:, :], xT[:, k * 128 : (k + 1) * 128], wbx_sb[i][:, :],
                        start=True, stop=False,
                    )
                    nc.tensor.matmul(
                        psy[:, :], ones[:, :], c_sb[:, :],
                        start=False, stop=True,
                    )
                    nc.scalar.copy(out=y_all[:, k * YS : k * YS + RNB], in_=psy[:, :])

                # Z.T (NB, E) = sum_{r, k} y'_r[k].T @ adjT_r[k]
                # per i-chunk n: accumulate Z chunk, then basis/highway tail on
                # ACT/DVE overlaps the next chunk's PE matmuls
                assert DR
                h_sb = workp.tile([HID, E], f32, tag="h", name=f"h_sb{i}", bufs=1)
                y_view = y_all[:, :].rearrange("p (k q) -> p k q", q=YQ)
                res_views = [
                    res_tiles[r][:, :].rearrange("p (c t i) -> p c t i", c=C2, t=2)
                    for r in range(R)
                ]
                for n in range(3):
                    ns = slice(n * NW, (n + 1) * NW)
                    psz = psB.tile([NB, 512], f32, tag="zz", bufs=2,
                                   name=f"psz{i}_{n}")
                    cnt = 0
                    for r in range(R):
                        for c in range(C2):
                            nc.tensor.matmul(
                                psz[:, 0:NW],
                                y_view[:, 2 * c : 2 * c + 2, 3 * r : 3 * r + 3],
                                res_views[r][:, c, :, ns],
                                start=(cnt == 0),
                                stop=(cnt == R * C2 - 1),
                                perf_mode=mybir.MatmulPerfMode.DoubleRow,
                            )
                            cnt += 1
                    z_sb = workp.tile([NB, NW], f32, tag="z_sb", bufs=2,
                                      name=f"z_sb{i}_{n}")
                    nc.scalar.copy(out=z_sb[:, :], in_=psz[:, 0:NW])
                    psh = psB.tile([HID, 512], f32, tag="hh", bufs=1,
                                   name=f"psh{i}_{n}")
                    nc.tensor.matmul(
                        psh[:, 0:NW], ww_sb[i][:, :], z_sb[:, :],
                        start=True, stop=True,
                    )
                    nc.scalar.activation(
                        h_sb[:, ns], psh[:, 0:NW], AF.Relu, bias=bias_sb[i][:, :],
                    )
                    psg = psB.tile([HID, 512], f32, tag="gg", bufs=1,
                                   name=f"psg{i}_{n}")
                    nc.tensor.matmul(
                        psg[:, 0:NW], wh_sb[i][:, :], h_sb[:, ns],
                        start=True, stop=True,
                    )
                    nc.scalar.activation(
                        psg[:, 0:NW], psg[:, 0:NW], AF.Sigmoid, bias=bh_sb[i][:, :],
                    )
                    # x = x + g * (h - x)  (chunk n)
                    nc.vector.tensor_sub(h_sb[:, ns], h_sb[:, ns], xT[:, ns])
                    nc.vector.tensor_mul(h_sb[:, ns], h_sb[:, ns], psg[:, 0:NW])
                    nc.vector.tensor_add(xT[:, ns], xT[:, ns], h_sb[:, ns])

            # ---- masked mean over entities ----
            xm = workp.tile([HID, E], f32, tag="h", name="xm", bufs=1)
            nc.vector.tensor_mul(xm[:, :], xT[:, 0:E], mask_sb[:, :])
            gsum = workp.tile([HID, 1], f32, tag="gsum", name="gsum", bufs=1)
            nc.vector.reduce_sum(gsum[:, :], xm[:, :], axis=AX.X)
            den = workp.tile([HID, 1], f32, tag="den", name="den", bufs=1)
            nc.vector.reduce_sum(den[:, :], mask_sb[:, :], axis=AX.X)
            nc.vector.tensor_scalar_max(den[:, :], den[:, :], 1.0)
            nc.vector.reciprocal(den[:, :], den[:, :])
            nc.vector.tensor_mul(gsum[:, :], gsum[:, :], den[:, :])
            nc.sync.dma_start(out=graphD, in_=gsum[:, :])

    nc.compile()
    return nc


def get_nc():
    if "nc" not in _NC_CACHE:
        _NC_CACHE["nc"] = _build_nc()
    return _NC_CACHE["nc"]


def make_in_maps(adj, mask_ids, ent_emb, rel_emb, Wb, Ww, bias, Wh, bh):
    adj = np.asarray(adj, dtype=np.float32)
    if DR:
        pad = np.zeros((B, R, EP, E2), dtype=ADT_NP)
        pad[:, :, :E, :E] = adj.transpose(0, 1, 3, 2).astype(ADT_NP)
        # [b, r, c, p, t, i] = adj[b, r, i, j = c*256 + t*128 + p]
        adjT = np.ascontiguousarray(
            pad.reshape(B, R, C2, 2, 128, E2).transpose(0, 1, 2, 4, 3, 5)
        )
    else:
        adjT = np.zeros((B, R, EP, E), dtype=ADT_NP)
        adjT[:, :, :E, :] = adj.transpose(0, 1, 3, 2).astype(ADT_NP)
    entT = np.ascontiguousarray(np.asarray(ent_emb, np.float32).T)
    relTh = np.ascontiguousarray(np.asarray(rel_emb, np.float32).T)
    Wb5 = np.asarray(Wb, np.float32).reshape(L, R, 2, D, NB)
    wbx = np.ascontiguousarray(Wb5[:, :, 0].transpose(0, 2, 1, 3).reshape(L, D, RNB))
    wbr = np.ascontiguousarray(Wb5[:, :, 1].transpose(0, 2, 1, 3).reshape(L, D, RNB))
    maskf = np.asarray(mask_ids).astype(np.float32)
    common = dict(
        xT0=entT, relT=relTh, wbx=wbx, wbr=wbr,
        ww=np.ascontiguousarray(np.asarray(Ww, np.float32)),
        wh=np.ascontiguousarray(np.asarray(Wh, np.float32)),
        biasL=np.ascontiguousarray(np.asarray(bias, np.float32)),
        bhL=np.ascontiguousarray(np.asarray(bh, np.float32)),
    )
    in_maps = []
    for c in range(8):
        b = c // 2
        m = dict(common)
        m["adjT"] = np.ascontiguousarray(adjT[b])
        m["maskrep"] = np.ascontiguousarray(
            np.broadcast_to(maskf[b][None, :], (HID, E))
        )
        in_maps.append(m)
    return in_maps


def run(inputs, trace=False):
    nc = get_nc()
    in_maps = make_in_maps(**{k: np.asarray(v) for k, v in inputs.items()})
    res = bass_utils.run_bass_kernel_spmd(
        nc, in_maps, core_ids=list(range(8)), trace=trace
    )
    out = np.stack(
        [np.asarray(res.results[2 * b]["graph"]).reshape(HID) for b in range(B)]
    ).astype(np.float32)
    return out, res


def kernel(**inputs):
    out, _ = run(inputs, trace=False)
    return out



# revision 10
# speedup vs baseline: 1.2050x; 1.2050x over previous
"""Trainium2 Bass kernel for the KGEncoder RGCN (nn_KGEncoder_14027363188782).

Math (per batch element b, L=5 layers):
    x0 = ent_emb                                             (E, D)
    per layer i:
      y_r   = x @ Wbx[i,r] + 1 * c[i,r]^T    (E, NB)  where c[i,r] = rel_r @ Wb_rel[i,r]
      Z     = sum_r adj_r @ y_r              (E, NB)  == sup @ Wb[i]
      h     = relu(Z @ Ww[i] + bias[i])
      g     = sigmoid(h @ Wh[i] + bh[i])
      x     = x + g * (h - x)
    out_b = sum_e x[e] * m[e] / max(sum_e m[e], 1)

Sharding: core c handles b = c // 2 (pair-replicated, no collectives).
adj shipped j-major (transposed) in fp8 (exact for 0/1), DoubleRow layout.
Host folds the constants: c[i] = rel @ Wb_rel[i] and y0 (layer-0 y, since
x0 == ent_emb is input-known).

Per layer the entity axis is split into 3 windows (512, 512, 476):
  Z.T(w) = sum_{r,c2} y.T @ adjT   (PE, fp8 DoubleRow)
  h(w)   = relu(Ww.T @ z + bias)   (PE bf16 + ACT)
  g(w)   = sigmoid(Wh.T @ h + bh)  (PE bf16 + ACT)
  x(w)  += g * (h - x)             (DVE)
Windows pipeline across engines; layer-0 Z accumulates relation-by-relation
behind the adj DMA stream; the last layer folds the masked mean into the
window tail.
"""

import numpy as np
import ml_dtypes

import concourse.bacc as bacc
import concourse.bass as bass
import concourse.mybir as mybir
import concourse.tile as tile
from concourse import bass_utils
from concourse.bass import MemorySpace

B, R, E, D, HID, L, NB = 4, 10, 1500, 100, 100, 5, 3
EP = 1536           # entity (j) dim padded to 12*128
CH = EP // 128      # 12 k-chunks
C2 = 6              # 256-row contraction chunks (DoubleRow)
E2 = 1504           # i dim padded to 16-aligned for DoubleRow strides
YQ = 32             # y_all per-chunk col stride (16-aligned)
RNB = R * NB        # 30
WINS = [(0, 512), (512, 512), (1024, 476)]   # (start, size) entity windows
f32 = mybir.dt.float32
bf16 = mybir.dt.bfloat16
fp8 = mybir.dt.float8e4
FP8_NP = ml_dtypes.float8_e4m3fn
AF = mybir.ActivationFunctionType
AX = mybir.AxisListType
DR = mybir.MatmulPerfMode.DoubleRow

_NC_CACHE = {}


def _build_nc():
    nc = bacc.Bacc("TRN2", target_bir_lowering=False, debug=False)

    adjT = nc.dram_tensor("adjT", [R, C2, 128, 2, E2], fp8, kind="ExternalInput").ap()
    xT0 = nc.dram_tensor("xT0", [D, E], f32, kind="ExternalInput").ap()
    maskrep = nc.dram_tensor("maskrep", [HID, E], f32, kind="ExternalInput").ap()
    y0qD = nc.dram_tensor("y0q", [128, CH * YQ], fp8, kind="ExternalInput").ap()
    callD = nc.dram_tensor("c_all", [1, L * RNB], f32, kind="ExternalInput").ap()
    wbxD = nc.dram_tensor("wbx", [L, D, RNB], f32, kind="ExternalInput").ap()
    wwD = nc.dram_tensor("wwb", [L, NB, HID], bf16, kind="ExternalInput").ap()
    whD = nc.dram_tensor("whb", [L, HID, HID], bf16, kind="ExternalInput").ap()
    biasD = nc.dram_tensor("biasL", [L, HID], f32, kind="ExternalInput").ap()
    bhD = nc.dram_tensor("bhL", [L, HID], f32, kind="ExternalInput").ap()
    graphD = nc.dram_tensor("graph", [HID, 1], f32, kind="ExternalOutput").ap()

    with tile.TileContext(nc) as tc:
        with (
            tc.tile_pool(name="singles", bufs=1) as singles,
            tc.tile_pool(name="resp", bufs=1) as resp,
            tc.tile_pool(name="ypool", bufs=2) as ypool,
            tc.tile_pool(name="hpool", bufs=2) as hpool,
            tc.tile_pool(name="zpool", bufs=3) as zpool,
            tc.tile_pool(name="tpool", bufs=2) as tpool,
            tc.tile_pool(name="psZ", bufs=3, space=MemorySpace.PSUM) as psZ,
            tc.tile_pool(name="psH", bufs=2, space=MemorySpace.PSUM) as psH,
            tc.tile_pool(name="psG", bufs=2, space=MemorySpace.PSUM) as psG,
            tc.tile_pool(name="psY", bufs=1, space=MemorySpace.PSUM) as psY,
        ):
            # ---- small persistent state (DMA-cheap, emitted first) ----
            ones = singles.tile([1, 128], f32, tag="ones", name="ones")
            nc.vector.memset(ones[:, :], 1.0)

            c_sb = singles.tile([1, L * RNB], f32, tag="c_all", name="c_sb")
            nc.sync.dma_start(out=c_sb[:, :], in_=callD)

            y0_sb = ypool.tile([128, CH * YQ], fp8, tag="y", name="y_all0")
            nc.sync.dma_start(out=y0_sb[:, :], in_=y0qD)

            wbx_sb, ww_sb, wh_sb, bias_sb, bh_sb = [], [], [], [], []
            for i in range(L):
                if i > 0:
                    wx = singles.tile([D, RNB], f32, tag=f"wbx{i}", name=f"wbx{i}")
                    nc.sync.dma_start(out=wx[:, :], in_=wbxD[i])
                else:
                    wx = None
                wbx_sb.append(wx)
                wwt = singles.tile([NB, HID], bf16, tag=f"ww{i}", name=f"ww{i}")
                nc.sync.dma_start(out=wwt[:, :], in_=wwD[i])
                ww_sb.append(wwt)
                wht = singles.tile([HID, HID], bf16, tag=f"wh{i}", name=f"wh{i}")
                nc.sync.dma_start(out=wht[:, :], in_=whD[i])
                wh_sb.append(wht)
                bt = singles.tile([HID, 1], f32, tag=f"bias{i}", name=f"bias{i}")
                nc.sync.dma_start(out=bt[:, :], in_=biasD[i].unsqueeze(1))
                bias_sb.append(bt)
                bht = singles.tile([HID, 1], f32, tag=f"bh{i}", name=f"bh{i}")
                nc.sync.dma_start(out=bht[:, :], in_=bhD[i].unsqueeze(1))
                bh_sb.append(bht)

            # ---- resident adjT (the big load; r-ordered so layer 0 chases it) ----
            res_tiles = []
            for r in range(R):
                rt = resp.tile([128, C2 * 2 * E2], fp8, tag=f"res{r}", name=f"res{r}")
                nc.sync.dma_start(
                    out=rt[:, :].rearrange("p (c t i) -> p c t i", c=C2, t=2),
                    in_=adjT[r].rearrange("c p t i -> p c t i"),
                )
                res_tiles.append(rt)
            res_views = [
                res_tiles[r][:, :].rearrange("p (c t i) -> p c t i", c=C2, t=2)
                for r in range(R)
            ]

            # ---- late DMAs (needed only from the layer-0 tail onwards) ----
            xT = singles.tile([D, EP], f32, tag="xT", name="xT")
            nc.sync.dma_start(out=xT[:, 0:E], in_=xT0)
            nc.vector.memset(xT[:, E:EP], 0.0)

            mask_sb = singles.tile([HID, E], f32, tag="mask", name="mask_sb")
            nc.sync.dma_start(out=mask_sb[:, :], in_=maskrep)

            # reciprocal of clamped mask sum (off critical path)
            rden = singles.tile([HID, 1], f32, tag="rden", name="rden")
            nc.vector.reduce_sum(rden[:, :], mask_sb[:, :], axis=AX.X)
            nc.vector.tensor_scalar_max(rden[:, :], rden[:, :], 1.0)
            nc.vector.reciprocal(rden[:, :], rden[:, :])

            gpart = singles.tile([HID, 4], f32, tag="gpart", name="gpart")

            # ---------------- layer bodies ----------------
            def z_mm_w(psz_w, w, y_view, r, c, start, stop):
                ws, wn = WINS[w]
                nc.tensor.matmul(
                    psz_w[:, 0:wn],
                    y_view[:, 2 * c: 2 * c + 2, 3 * r: 3 * r + 3],
                    res_views[r][:, c, :, ws: ws + wn],
                    start=start, stop=stop,
                    perf_mode=DR,
                )

            def psy_chunk(i, k, y_all):
                """y[:, k-chunk] = x[:, k-chunk] @ Wbx[i] + 1 (x) c[i]"""
                psy = psY.tile([128, RNB], f32, tag="y", name=f"psy{i}_{k}")
                nc.tensor.matmul(
                    psy[:, :], xT[:, k * 128:(k + 1) * 128], wbx_sb[i][:, :],
                    start=True, stop=False,
                )
                nc.tensor.matmul(
                    psy[:, :], ones[:, :],
                    c_sb[:, i * RNB:(i + 1) * RNB],
                    start=False, stop=True,
                )
                nc.scalar.copy(
                    out=y_all[:, k * YQ: k * YQ + RNB], in_=psy[:, :]
                )

            def tail_zcopy(i, w, psz_w, z_sb):
                ws, wn = WINS[w]
                nc.scalar.copy(out=z_sb[:, 0:wn], in_=psz_w[:, 0:wn])

            def tail_psh(i, w, z_sb, psh_w):
                ws, wn = WINS[w]
                nc.tensor.matmul(
                    psh_w[:, 0:wn], ww_sb[i][:, :], z_sb[:, 0:wn],
                    start=True, stop=True,
                )

            def tail_relu(i, w, psh_w, h_sb):
                ws, wn = WINS[w]
                nc.scalar.activation(
                    h_sb[:, ws:ws + wn], psh_w[:, 0:wn], AF.Relu,
                    bias=bias_sb[i][:, :],
                )

            def tail_psg(i, w, h_sb, psg_w):
                ws, wn = WINS[w]
                nc.tensor.matmul(
                    psg_w[:, 0:wn], wh_sb[i][:, :], h_sb[:, ws:ws + wn],
                    start=True, stop=True,
                )

            def tail_x(i, w, h_sb, psg_w):
                """sigmoid on ACT, then x += g*(h-x) on DVE.
                Last layer: also fold the masked partial sum."""
                ws, wn = WINS[w]
                nc.scalar.activation(
                    psg_w[:, 0:wn], psg_w[:, 0:wn], AF.Sigmoid,
                    bias=bh_sb[i][:, :],
                )
                t = tpool.tile([HID, 512], f32, tag="t", name=f"t{i}_{w}")
                nc.vector.tensor_sub(t[:, 0:wn], h_sb[:, ws:ws + wn], xT[:, ws:ws + wn])
                nc.vector.tensor_mul(t[:, 0:wn], t[:, 0:wn], psg_w[:, 0:wn])
                if i < L - 1:
                    nc.vector.tensor_add(xT[:, ws:ws + wn], xT[:, ws:ws + wn], t[:, 0:wn])
                else:
                    nc.vector.tensor_add(t[:, 0:wn], xT[:, ws:ws + wn], t[:, 0:wn])
                    nc.vector.tensor_mul(t[:, 0:wn], t[:, 0:wn], mask_sb[:, ws:ws + wn])
                    nc.vector.reduce_sum(gpart[:, w:w + 1], t[:, 0:wn], axis=AX.X)

            # ---- layer 0: relation-outer, chasing the adj DMA stream ----
            y_view0 = y0_sb[:, :].rearrange("p (k q) -> p k q", q=YQ)
            psz0 = [
                psZ.tile([NB, 512], f32, tag="z", name=f"psz0_{w}") for w in range(3)
            ]
            for w in range(3):
                for r in range(R):
                    for c in range(C2):
                        z_mm_w(psz0[w], w, y_view0, r, c,
                               start=(r == 0 and c == 0),
                               stop=(r == R - 1 and c == C2 - 1))
            # tails for layer 0 (+ y chunks for layer 1 interleaved)
            h0 = hpool.tile([HID, E], bf16, tag="h", name="h0")
            z0 = [zpool.tile([NB, 512], bf16, tag="z", name=f"z0_{w}") for w in range(3)]
            ps_h0 = [psH.tile([HID, 512], f32, tag="h", name=f"psh0_{w}") for w in range(2)]
            ps_g0 = [psG.tile([HID, 512], f32, tag="g", name=f"psg0_{w}") for w in range(2)]
            y1 = ypool.tile([128, CH * YQ], fp8, tag="y", name="y_all1")

            tail_zcopy(0, 0, psz0[0], z0[0])
            tail_psh(0, 0, z0[0], ps_h0[0])
            tail_relu(0, 0, ps_h0[0], h0)
            tail_zcopy(0, 1, psz0[1], z0[1])
            tail_psg(0, 0, h0, ps_g0[0])
            tail_psh(0, 1, z0[1], ps_h0[1])
            tail_x(0, 0, h0, ps_g0[0])
            tail_relu(0, 1, ps_h0[1], h0)
            tail_zcopy(0, 2, psz0[2], z0[2])
            tail_psg(0, 1, h0, ps_g0[1])
            ps_h2 = psH.tile([HID, 512], f32, tag="h", name="psh0_2")
            tail_psh(0, 2, z0[2], ps_h2)
            tail_x(0, 1, h0, ps_g0[1])
            tail_relu(0, 2, ps_h2, h0)
            for k in range(4):
                psy_chunk(1, k, y1)
            ps_g2 = psG.tile([HID, 512], f32, tag="g", name="psg0_2")
            tail_psg(0, 2, h0, ps_g2)
            for k in range(4, 8):
                psy_chunk(1, k, y1)
            tail_x(0, 2, h0, ps_g2)
            for k in range(8, CH):
                psy_chunk(1, k, y1)

            # ---- layers 1..4: window-pipelined, c2-ordered for boundary overlap ----
            prev_y = y1
            for i in range(1, L):
                y_view = prev_y[:, :].rearrange("p (k q) -> p k q", q=YQ)
                psz = [psZ.tile([NB, 512], f32, tag="z", name=f"psz{i}_{w}")
                       for w in range(3)]
                z_sb = [zpool.tile([NB, 512], bf16, tag="z", name=f"z{i}_{w}")
                        for w in range(3)]
                h_sb = hpool.tile([HID, E], bf16, tag="h", name=f"h{i}")
                ps_h = [psH.tile([HID, 512], f32, tag="h", name=f"psh{i}_{w}")
                        for w in range(2)]
                ps_g = [psG.tile([HID, 512], f32, tag="g", name=f"psg{i}_{w}")
                        for w in range(2)]
                if i < L - 1:
                    next_y = ypool.tile([128, CH * YQ], fp8, tag="y", name=f"y_all{i+1}")

                def zw(w, cs, stop_c=C2 - 1):
                    for c in cs:
                        for r in range(R):
                            z_mm_w(psz[w], w, y_view, r, c,
                                   start=(c == 0 and r == 0),
                                   stop=(c == stop_c and r == R - 1))

                # window 0: c2 blocks whose y-chunks are ready first
                zw(0, [0, 1, 2, 3, 4, 5])
                zw(1, [0, 1, 2, 3, 4, 5])
                tail_zcopy(i, 0, psz[0], z_sb[0])
                tail_psh(i, 0, z_sb[0], ps_h[0])
                zw(2, [0, 1, 2])
                tail_relu(i, 0, ps_h[0], h_sb)
                tail_psg(i, 0, h_sb, ps_g[0])
                zw(2, [3, 4, 5])
                tail_x(i, 0, h_sb, ps_g[0])
                tail_zcopy(i, 1, psz[1], z_sb[1])
                tail_psh(i, 1, z_sb[1], ps_h[1])
                tail_relu(i, 1, ps_h[1], h_sb)
                tail_psg(i, 1, h_sb, ps_g[1])
                tail_x(i, 1, h_sb, ps_g[1])
                tail_zcopy(i, 2, psz[2], z_sb[2])
                ps_h2i = psH.tile([HID, 512], f32, tag="h", name=f"psh{i}_2")
                tail_psh(i, 2, z_sb[2], ps_h2i)
                tail_relu(i, 2, ps_h2i, h_sb)
                if i < L - 1:
                    for k in range(4):
                        psy_chunk(i + 1, k, next_y)
                ps_g2i = psG.tile([HID, 512], f32, tag="g", name=f"psg{i}_2")
                tail_psg(i, 2, h_sb, ps_g2i)
                if i < L - 1:
                    for k in range(4, 8):
                        psy_chunk(i + 1, k, next_y)
                tail_x(i, 2, h_sb, ps_g2i)
                if i < L - 1:
                    for k in range(8, CH):
                        psy_chunk(i + 1, k, next_y)
                    prev_y = next_y

            # ---- masked mean epilogue (partials already in gpart) ----
            gsum = singles.tile([HID, 1], f32, tag="gsum", name="gsum")
            nc.vector.reduce_sum(gsum[:, :], gpart[:, 0:3], axis=AX.X)
            nc.vector.tensor_mul(gsum[:, :], gsum[:, :], rden[:, :])
            nc.sync.dma_start(out=graphD, in_=gsum[:, :])

    nc.compile()
    return nc


def get_nc():
    if "nc" not in _NC_CACHE:
        _NC_CACHE["nc"] = _build_nc()
    return _NC_CACHE["nc"]


def make_in_maps(adj, mask_ids, ent_emb, rel_emb, Wb, Ww, bias, Wh, bh):
    adj = np.asarray(adj, dtype=np.float32)
    pad = np.zeros((B, R, EP, E2), dtype=FP8_NP)
    pad[:, :, :E, :E] = adj.transpose(0, 1, 3, 2).astype(FP8_NP)
    # [b, r, c, p, t, i] = adj[b, r, i, j = c*256 + t*128 + p]
    adjT = np.ascontiguousarray(
        pad.reshape(B, R, C2, 2, 128, E2).transpose(0, 1, 2, 4, 3, 5)
    )
    ent = np.asarray(ent_emb, np.float32)
    entT = np.ascontiguousarray(ent.T)
    relf = np.asarray(rel_emb, np.float32)
    Wb5 = np.asarray(Wb, np.float32).reshape(L, R, 2, D, NB)
    wbx = np.ascontiguousarray(Wb5[:, :, 0].transpose(0, 2, 1, 3).reshape(L, D, RNB))
    # c[i, 3r+nb] = rel_r @ Wb_rel[i, r]
    c_all = np.einsum("rd,irdn->irn", relf, Wb5[:, :, 1]).reshape(L, RNB)
    # layer-0 y (x0 == ent_emb): (EP, RNB), pad rows get the c constant
    y0 = np.broadcast_to(c_all[0][None, :], (EP, RNB)).copy()
    y0[:E] += ent @ wbx[0]
    y0q = np.zeros((128, CH * YQ), dtype=FP8_NP)
    y0q_view = y0q.reshape(128, CH, YQ)
    y0_k = y0.reshape(CH, 128, RNB).transpose(1, 0, 2)   # (p, k, RNB)
    y0q_view[:, :, :RNB] = y0_k.astype(FP8_NP)
    maskf = np.asarray(mask_ids).astype(np.float32)
    common = dict(
        xT0=entT,
        y0q=y0q,
        c_all=np.ascontiguousarray(c_all.reshape(1, L * RNB)),
        wbx=wbx,
        wwb=np.ascontiguousarray(np.asarray(Ww, np.float32).astype(ml_dtypes.bfloat16)),
        whb=np.ascontiguousarray(np.asarray(Wh, np.float32).astype(ml_dtypes.bfloat16)),
        biasL=np.ascontiguousarray(np.asarray(bias, np.float32)),
        bhL=np.ascontiguousarray(np.asarray(bh, np.float32)),
    )
    in_maps = []
    for c in range(8):
        b = c // 2
        m = dict(common)
        m["adjT"] = np.ascontiguousarray(adjT[b])
        m["maskrep"] = np.ascontiguousarray(
            np.broadcast_to(maskf[b][None, :], (HID, E))
        )
        in_maps.append(m)
    return in_maps


def run(inputs, trace=False):
    nc = get_nc()
    in_maps = make_in_maps(**{k: np.asarray(v) for k, v in inputs.items()})
    res = bass_utils.run_bass_kernel_spmd(
        nc, in_maps, core_ids=list(range(8)), trace=trace
    )
    out = np.stack(
        [np.asarray(res.results[2 * b]["graph"]).reshape(HID) for b in range(B)]
    ).astype(np.float32)
    return out, res


def kernel(**inputs):
    out, _ = run(inputs, trace=False)
    return out


# revision 15
# speedup vs baseline: 1.3062x; 1.0839x over previous
"""Trainium2 Bass kernel for the KGEncoder RGCN (nn_KGEncoder_14027363188782).

Math (per batch element b, L=5 layers):
    x0 = ent_emb                                             (E, D)
    per layer i:
      y_r   = x @ Wbx[i,r] + 1 * c[i,r]^T    (E, NB)  where c[i,r] = rel_r @ Wb_rel[i,r]
      Z     = sum_r adj_r @ y_r              (E, NB)  == sup @ Wb[i]
      h     = relu(Z @ Ww[i] + bias[i])
      g     = sigmoid(h @ Wh[i] + bh[i])
      x     = x + g * (h - x)
    out_b = sum_e x[e] * m[e] / max(sum_e m[e], 1)

Sharding: core c handles b = c // 2 (pair-replicated, no collectives).
adj shipped j-major (transposed) in fp8 (exact for 0/1), DoubleRow layout.
Host folds the constants: c[i] = rel @ Wb_rel[i] and y0 (layer-0 y, since
x0 == ent_emb is input-known).

Per layer the entity axis is split into 3 windows (512, 512, 476):
  Z.T(w) = sum_{r,c2} y.T @ adjT   (PE, fp8 DoubleRow)
  h(w)   = relu(Ww.T @ z + bias)   (PE bf16 + ACT)
  g(w)   = sigmoid(Wh.T @ h + bh)  (PE bf16 + ACT)
  x(w)  += g * (h - x)             (DVE)
Windows pipeline across engines; layer-0 Z accumulates relation-by-relation
behind the adj DMA stream; the last layer folds the masked mean into the
window tail.
"""

import numpy as np
import ml_dtypes

import concourse.bacc as bacc
import concourse.bass as bass
import concourse.mybir as mybir
import concourse.tile as tile
from concourse import bass_utils
from concourse.bass import MemorySpace

B, R, E, D, HID, L, NB = 4, 10, 1500, 100, 100, 5, 3
EP = 1536           # entity (j) dim padded to 12*128
CH = EP // 128      # 12 k-chunks
C2 = 6              # 256-row contraction chunks (DoubleRow)
E2 = 1504           # i dim padded to 16-aligned for DoubleRow strides
YQ = 32             # y_all per-chunk col stride (16-aligned)
RNB = R * NB        # 30
WINS = [(0, 512), (512, 512), (1024, 476)]   # (start, size) entity windows
f32 = mybir.dt.float32
bf16 = mybir.dt.bfloat16
fp8 = mybir.dt.float8e4
FP8_NP = ml_dtypes.float8_e4m3fn
AF = mybir.ActivationFunctionType
AX = mybir.AxisListType
DR = mybir.MatmulPerfMode.DoubleRow

_NC_CACHE = {}


def _build_nc():
    nc = bacc.Bacc("TRN2", target_bir_lowering=False, debug=False)

    adjT = nc.dram_tensor("adjT", [R, C2, 128, 2, E2], fp8, kind="ExternalInput").ap()
    xT0 = nc.dram_tensor("xT0", [D, E], f32, kind="ExternalInput").ap()
    maskrep = nc.dram_tensor("maskrep", [HID, E], f32, kind="ExternalInput").ap()
    y0qD = nc.dram_tensor("y0q", [128, CH * YQ], fp8, kind="ExternalInput").ap()
    # packed weight blobs (single DMA each):
    # wblob bf16 (128, L*200): per layer i, cols [200i,200i+100) rows 0:NB = Ww[i],
    #   cols [200i+100,200i+200) rows 0:HID = Wh[i]
    # fblob f32 (128, 280): col i rows 0:HID = bias[i]; col 5+i = bh[i];
    #   cols [10+30(i-1),...) rows 0:D = wbx[i] (i>=1); row 0 cols 130..280 = c_all
    wblobD = nc.dram_tensor("wblob", [128, L * 200], bf16, kind="ExternalInput").ap()
    fblobD = nc.dram_tensor("fblob", [128, 280], f32, kind="ExternalInput").ap()
    graphD = nc.dram_tensor("graph", [HID, 1], f32, kind="ExternalOutput").ap()

    with tile.TileContext(nc) as tc:
        with (
            tc.tile_pool(name="singles", bufs=1) as singles,
            tc.tile_pool(name="resp", bufs=1) as resp,
            tc.tile_pool(name="ypool", bufs=2) as ypool,
            tc.tile_pool(name="hpool", bufs=2) as hpool,
            tc.tile_pool(name="zpool", bufs=3) as zpool,
            tc.tile_pool(name="tpool", bufs=2) as tpool,
            tc.tile_pool(name="psZ", bufs=3, space=MemorySpace.PSUM) as psZ,
            tc.tile_pool(name="psH", bufs=2, space=MemorySpace.PSUM) as psH,
            tc.tile_pool(name="psG", bufs=2, space=MemorySpace.PSUM) as psG,
            tc.tile_pool(name="psY", bufs=1, space=MemorySpace.PSUM) as psY,
        ):
            # ---- y0 first (layer 0 needs it immediately), then the big adj load ----
            ones = singles.tile([1, 128], f32, tag="ones", name="ones")
            nc.vector.memset(ones[:, :], 1.0)

            y0_sb = ypool.tile([128, CH * YQ], fp8, tag="y", name="y_all0")
            nc.sync.dma_start(out=y0_sb[:, :], in_=y0qD)

            # ---- resident adjT (the big load; r-ordered so layer 0 chases it) ----
            res_tiles = []
            for r in range(R):
                rt = resp.tile([128, C2 * 2 * E2], fp8, tag=f"res{r}", name=f"res{r}")
                nc.sync.dma_start(
                    out=rt[:, :].rearrange("p (c t i) -> p c t i", c=C2, t=2),
                    in_=adjT[r].rearrange("c p t i -> p c t i"),
                )
                res_tiles.append(rt)
            res_views = [
                res_tiles[r][:, :].rearrange("p (c t i) -> p c t i", c=C2, t=2)
                for r in range(R)
            ]

            # ---- late DMAs (needed only from the layer-0 tail onwards) ----
            wblob = singles.tile([128, L * 200], bf16, tag="wblob", name="wblob")
            nc.sync.dma_start(out=wblob[:, :], in_=wblobD)
            fblob = singles.tile([128, 280], f32, tag="fblob", name="fblob")
            nc.sync.dma_start(out=fblob[:, :], in_=fblobD)
            ww_sb = [wblob[0:NB, 200 * i: 200 * i + HID] for i in range(L)]
            wh_sb = [wblob[0:HID, 200 * i + 100: 200 * i + 200] for i in range(L)]
            bias_sb = [fblob[0:HID, i:i + 1] for i in range(L)]
            bh_sb = [fblob[0:HID, 5 + i: 6 + i] for i in range(L)]
            wbx_sb = [None] + [
                fblob[0:D, 10 + 30 * (i - 1): 10 + 30 * i] for i in range(1, L)
            ]
            c_views = [fblob[0:1, 130 + i * RNB: 130 + (i + 1) * RNB] for i in range(L)]

            xT = singles.tile([D, EP], f32, tag="xT", name="xT")
            nc.sync.dma_start(out=xT[:, 0:E], in_=xT0)
            nc.vector.memset(xT[:, E:EP], 0.0)

            mask_sb = singles.tile([HID, E], f32, tag="mask", name="mask_sb")
            nc.sync.dma_start(out=mask_sb[:, :], in_=maskrep)

            # reciprocal of clamped mask sum (off critical path)
            rden = singles.tile([HID, 1], f32, tag="rden", name="rden")
            nc.vector.reduce_sum(rden[:, :], mask_sb[:, :], axis=AX.X)
            nc.vector.tensor_scalar_max(rden[:, :], rden[:, :], 1.0)
            nc.vector.reciprocal(rden[:, :], rden[:, :])

            gpart = singles.tile([HID, 4], f32, tag="gpart", name="gpart")

            # ---------------- layer bodies ----------------
            def z_mm_w(psz_w, w, y_view, r, c, start, stop):
                ws, wn = WINS[w]
                nc.tensor.matmul(
                    psz_w[:, 0:wn],
                    y_view[:, 2 * c: 2 * c + 2, 3 * r: 3 * r + 3],
                    res_views[r][:, c, :, ws: ws + wn],
                    start=start, stop=stop,
                    perf_mode=DR,
                )

            def psy_chunk(i, k, y_all):
                """y[:, k-chunk] = x[:, k-chunk] @ Wbx[i] + 1 (x) c[i]"""
                psy = psY.tile([128, RNB], f32, tag="y", name=f"psy{i}_{k}")
                nc.tensor.matmul(
                    psy[:, :], xT[:, k * 128:(k + 1) * 128], wbx_sb[i],
                    start=True, stop=False,
                )
                nc.tensor.matmul(
                    psy[:, :], ones[:, :],
                    c_views[i],
                    start=False, stop=True,
                )
                nc.scalar.copy(
                    out=y_all[:, k * YQ: k * YQ + RNB], in_=psy[:, :]
                )

            def tail_zcopy(i, w, psz_w, z_sb):
                ws, wn = WINS[w]
                nc.scalar.copy(out=z_sb[:, 0:wn], in_=psz_w[:, 0:wn])

            def tail_psh(i, w, z_sb, psh_w):
                ws, wn = WINS[w]
                nc.tensor.matmul(
                    psh_w[:, 0:wn], ww_sb[i], z_sb[:, 0:wn],
                    start=True, stop=True,
                )

            def tail_relu(i, w, psh_w, h_sb):
                ws, wn = WINS[w]
                nc.scalar.activation(
                    h_sb[:, ws:ws + wn], psh_w[:, 0:wn], AF.Relu,
                    bias=bias_sb[i],
                )

            def tail_psg(i, w, h_sb, psg_w):
                ws, wn = WINS[w]
                nc.tensor.matmul(
                    psg_w[:, 0:wn], wh_sb[i], h_sb[:, ws:ws + wn],
                    start=True, stop=True,
                )

            def tail_x(i, w, h_sb, psg_w):
                """sigmoid on ACT, then x += g*(h-x) on DVE.
                Last layer: also fold the masked partial sum."""
                ws, wn = WINS[w]
                nc.scalar.activation(
                    psg_w[:, 0:wn], psg_w[:, 0:wn], AF.Sigmoid,
                    bias=bh_sb[i],
                )
                t = tpool.tile([HID, 512], f32, tag="t", name=f"t{i}_{w}")
                nc.vector.tensor_sub(t[:, 0:wn], h_sb[:, ws:ws + wn], xT[:, ws:ws + wn])
                nc.vector.tensor_mul(t[:, 0:wn], t[:, 0:wn], psg_w[:, 0:wn])
                if i < L - 1:
                    nc.vector.tensor_add(xT[:, ws:ws + wn], xT[:, ws:ws + wn], t[:, 0:wn])
                else:
                    nc.vector.tensor_add(t[:, 0:wn], xT[:, ws:ws + wn], t[:, 0:wn])
                    nc.vector.tensor_mul(t[:, 0:wn], t[:, 0:wn], mask_sb[:, ws:ws + wn])
                    nc.vector.reduce_sum(gpart[:, w:w + 1], t[:, 0:wn], axis=AX.X)

            # ---- layer 0: relation-outer, chasing the adj DMA stream ----
            y_view0 = y0_sb[:, :].rearrange("p (k q) -> p k q", q=YQ)
            psz0 = [
                psZ.tile([NB, 512], f32, tag="z", name=f"psz0_{w}") for w in range(3)
            ]
            for w in range(3):
                for r in range(R):
                    for c in range(C2):
                        z_mm_w(psz0[w], w, y_view0, r, c,
                               start=(r == 0 and c == 0),
                               stop=(r == R - 1 and c == C2 - 1))
            # tails for layer 0 (+ y chunks for layer 1 interleaved)
            h0 = hpool.tile([HID, E], bf16, tag="h", name="h0")
            z0 = [zpool.tile([NB, 512], bf16, tag="z", name=f"z0_{w}") for w in range(3)]
            ps_h0 = [psH.tile([HID, 512], f32, tag="h", name=f"psh0_{w}") for w in range(2)]
            ps_g0 = [psG.tile([HID, 512], f32, tag="g", name=f"psg0_{w}") for w in range(2)]
            y1 = ypool.tile([128, CH * YQ], fp8, tag="y", name="y_all1")

            tail_zcopy(0, 0, psz0[0], z0[0])
            tail_psh(0, 0, z0[0], ps_h0[0])
            tail_relu(0, 0, ps_h0[0], h0)
            tail_zcopy(0, 1, psz0[1], z0[1])
            tail_psg(0, 0, h0, ps_g0[0])
            tail_psh(0, 1, z0[1], ps_h0[1])
            tail_x(0, 0, h0, ps_g0[0])
            tail_relu(0, 1, ps_h0[1], h0)
            tail_zcopy(0, 2, psz0[2], z0[2])
            tail_psg(0, 1, h0, ps_g0[1])
            ps_h2 = psH.tile([HID, 512], f32, tag="h", name="psh0_2")
            tail_psh(0, 2, z0[2], ps_h2)
            tail_x(0, 1, h0, ps_g0[1])
            tail_relu(0, 2, ps_h2, h0)
            for k in range(4):
                psy_chunk(1, k, y1)
            ps_g2 = psG.tile([HID, 512], f32, tag="g", name="psg0_2")
            tail_psg(0, 2, h0, ps_g2)
            for k in range(4, 8):
                psy_chunk(1, k, y1)
            tail_x(0, 2, h0, ps_g2)
            for k in range(8, CH):
                psy_chunk(1, k, y1)

            # ---- layers 1..4: window-pipelined, c2-ordered for boundary overlap ----
            prev_y = y1
            for i in range(1, L):
                y_view = prev_y[:, :].rearrange("p (k q) -> p k q", q=YQ)
                psz = [psZ.tile([NB, 512], f32, tag="z", name=f"psz{i}_{w}")
                       for w in range(3)]
                z_sb = [zpool.tile([NB, 512], bf16, tag="z", name=f"z{i}_{w}")
                        for w in range(3)]
                h_sb = hpool.tile([HID, E], bf16, tag="h", name=f"h{i}")
                ps_h = [psH.tile([HID, 512], f32, tag="h", name=f"psh{i}_{w}")
                        for w in range(2)]
                ps_g = [psG.tile([HID, 512], f32, tag="g", name=f"psg{i}_{w}")
                        for w in range(2)]
                if i < L - 1:
                    next_y = ypool.tile([128, CH * YQ], fp8, tag="y", name=f"y_all{i+1}")

                def zw(w, cs, stop_c=C2 - 1):
                    for c in cs:
                        for r in range(R):
                            z_mm_w(psz[w], w, y_view, r, c,
                                   start=(c == 0 and r == 0),
                                   stop=(c == stop_c and r == R - 1))

                # window 0: c2 blocks whose y-chunks are ready first
                zw(0, [0, 1, 2, 3, 4, 5])
                zw(1, [0, 1, 2, 3, 4, 5])
                tail_zcopy(i, 0, psz[0], z_sb[0])
                tail_psh(i, 0, z_sb[0], ps_h[0])
                zw(2, [0, 1, 2])
                tail_relu(i, 0, ps_h[0], h_sb)
                tail_psg(i, 0, h_sb, ps_g[0])
                zw(2, [3, 4, 5])
                tail_x(i, 0, h_sb, ps_g[0])
                tail_zcopy(i, 1, psz[1], z_sb[1])
                tail_psh(i, 1, z_sb[1], ps_h[1])
                tail_relu(i, 1, ps_h[1], h_sb)
                tail_psg(i, 1, h_sb, ps_g[1])
                tail_x(i, 1, h_sb, ps_g[1])
                tail_zcopy(i, 2, psz[2], z_sb[2])
                ps_h2i = psH.tile([HID, 512], f32, tag="h", name=f"psh{i}_2")
                tail_psh(i, 2, z_sb[2], ps_h2i)
                tail_relu(i, 2, ps_h2i, h_sb)
                if i < L - 1:
                    for k in range(4):
                        psy_chunk(i + 1, k, next_y)
                ps_g2i = psG.tile([HID, 512], f32, tag="g", name=f"psg{i}_2")
                tail_psg(i, 2, h_sb, ps_g2i)
                if i < L - 1:
                    for k in range(4, 8):
                        psy_chunk(i + 1, k, next_y)
                tail_x(i, 2, h_sb, ps_g2i)
                if i < L - 1:
                    for k in range(8, CH):
                        psy_chunk(i + 1, k, next_y)
                    prev_y = next_y

            # ---- masked mean epilogue (partials already in gpart) ----
            gsum = singles.tile([HID, 1], f32, tag="gsum", name="gsum")
            nc.vector.reduce_sum(gsum[:, :], gpart[:, 0:3], axis=AX.X)
            nc.vector.tensor_mul(gsum[:, :], gsum[:, :], rden[:, :])
            nc.sync.dma_start(out=graphD, in_=gsum[:, :])

    nc.compile()
    return nc


def get_nc():
    if "nc" not in _NC_CACHE:
        _NC_CACHE["nc"] = _build_nc()
    return _NC_CACHE["nc"]


def make_in_maps(adj, mask_ids, ent_emb, rel_emb, Wb, Ww, bias, Wh, bh):
    adj = np.asarray(adj, dtype=np.float32)
    pad = np.zeros((B, R, EP, E2), dtype=FP8_NP)
    pad[:, :, :E, :E] = adj.transpose(0, 1, 3, 2).astype(FP8_NP)
    # [b, r, c, p, t, i] = adj[b, r, i, j = c*256 + t*128 + p]
    adjT = np.ascontiguousarray(
        pad.reshape(B, R, C2, 2, 128, E2).transpose(0, 1, 2, 4, 3, 5)
    )
    ent = np.asarray(ent_emb, np.float32)
    entT = np.ascontiguousarray(ent.T)
    relf = np.asarray(rel_emb, np.float32)
    Wb5 = np.asarray(Wb, np.float32).reshape(L, R, 2, D, NB)
    wbx = np.ascontiguousarray(Wb5[:, :, 0].transpose(0, 2, 1, 3).reshape(L, D, RNB))
    # c[i, 3r+nb] = rel_r @ Wb_rel[i, r]
    c_all = np.einsum("rd,irdn->irn", relf, Wb5[:, :, 1]).reshape(L, RNB)
    # layer-0 y (x0 == ent_emb): (EP, RNB), pad rows get the c constant
    y0 = np.broadcast_to(c_all[0][None, :], (EP, RNB)).copy()
    y0[:E] += ent @ wbx[0]
    y0q = np.zeros((128, CH * YQ), dtype=FP8_NP)
    y0q_view = y0q.reshape(128, CH, YQ)
    y0_k = y0.reshape(CH, 128, RNB).transpose(1, 0, 2)   # (p, k, RNB)
    y0q_view[:, :, :RNB] = y0_k.astype(FP8_NP)
    maskf = np.asarray(mask_ids).astype(np.float32)
    wwb = np.asarray(Ww, np.float32).astype(ml_dtypes.bfloat16)
    whb = np.asarray(Wh, np.float32).astype(ml_dtypes.bfloat16)
    wblob = np.zeros((128, L * 200), dtype=ml_dtypes.bfloat16)
    fblob = np.zeros((128, 280), dtype=np.float32)
    biasf = np.asarray(bias, np.float32)
    bhf = np.asarray(bh, np.float32)
    for i in range(L):
        wblob[0:NB, 200 * i: 200 * i + HID] = wwb[i]
        wblob[0:HID, 200 * i + 100: 200 * i + 200] = whb[i]
        fblob[0:HID, i] = biasf[i]
        fblob[0:HID, 5 + i] = bhf[i]
        if i >= 1:
            fblob[0:D, 10 + 30 * (i - 1): 10 + 30 * i] = wbx[i]
    fblob[0, 130: 130 + L * RNB] = c_all.reshape(-1)
    common = dict(
        xT0=entT,
        y0q=y0q,
        wblob=wblob,
        fblob=fblob,
    )
    in_maps = []
    for c in range(8):
        b = c // 2
        m = dict(common)
        m["adjT"] = np.ascontiguousarray(adjT[b])
        m["maskrep"] = np.ascontiguousarray(
            np.broadcast_to(maskf[b][None, :], (HID, E))
        )
        in_maps.append(m)
    return in_maps


def run(inputs, trace=False):
    nc = get_nc()
    in_maps = make_in_maps(**{k: np.asarray(v) for k, v in inputs.items()})
    res = bass_utils.run_bass_kernel_spmd(
        nc, in_maps, core_ids=list(range(8)), trace=trace
    )
    out = np.stack(
        [np.asarray(res.results[2 * b]["graph"]).reshape(HID) for b in range(B)]
    ).astype(np.float32)
    return out, res


def kernel(**inputs):
    out, _ = run(inputs, trace=False)
    return out


# revision 16
# speedup vs baseline: 1.5987x; 1.2240x over previous
"""Trainium2 Bass kernel for the KGEncoder RGCN (nn_KGEncoder_14027363188782).

Math (per batch element b, L=5 layers):
    x0 = ent_emb                                             (E, D)
    per layer i:
      y_r   = x @ Wbx[i,r] + 1 * c[i,r]^T    (E, NB)  where c[i,r] = rel_r @ Wb_rel[i,r]
      Z     = sum_r adj_r @ y_r              (E, NB)  == sup @ Wb[i]
      h     = relu(Z @ Ww[i] + bias[i])
      g     = sigmoid(h @ Wh[i] + bh[i])
      x     = x + g * (h - x)
    out_b = sum_e x[e] * m[e] / max(sum_e m[e], 1)

Sharding: core c handles b = c // 2 (pair-replicated, no collectives).
adj shipped j-major (transposed) in fp8 (exact for 0/1), DoubleRow layout.
Host folds the constants: c[i] = rel @ Wb_rel[i] and y0 (layer-0 y, since
x0 == ent_emb is input-known).

Per layer the entity axis is split into 3 windows (512, 512, 476):
  Z.T(w) = sum_{r,c2} y.T @ adjT   (PE, fp8 DoubleRow)
  h(w)   = relu(Ww.T @ z + bias)   (PE bf16 + ACT)
  g(w)   = sigmoid(Wh.T @ h + bh)  (PE bf16 + ACT)
  x(w)  += g * (h - x)             (DVE)
Windows pipeline across engines; layer-0 Z accumulates relation-by-relation
behind the adj DMA stream; the last layer folds the masked mean into the
window tail.
"""

import numpy as np
import ml_dtypes

import concourse.bacc as bacc
import concourse.bass as bass
import concourse.mybir as mybir
import concourse.tile as tile
from concourse import bass_utils
from concourse.bass import MemorySpace

B, R, E, D, HID, L, NB = 4, 10, 1500, 100, 100, 5, 3
EP = 1536           # entity (j) dim padded to 12*128
CH = EP // 128      # 12 k-chunks
C2 = 6              # 256-row contraction chunks (DoubleRow)
E2 = 1504           # i dim padded to 16-aligned for DoubleRow strides
YQ = 32             # y_all per-chunk col stride (16-aligned)
RNB = R * NB        # 30
WINS = [(0, 512), (512, 512), (1024, 476)]   # (start, size) entity windows
f32 = mybir.dt.float32
bf16 = mybir.dt.bfloat16
fp8 = mybir.dt.float8e4
FP8_NP = ml_dtypes.float8_e4m3fn
AF = mybir.ActivationFunctionType
AX = mybir.AxisListType
DR = mybir.MatmulPerfMode.DoubleRow

_NC_CACHE = {}


def _build_nc():
    nc = bacc.Bacc("TRN2", target_bir_lowering=False, debug=False)

    adjT = nc.dram_tensor("adjT", [R, C2, 128, 2, E2], fp8, kind="ExternalInput").ap()
    xT0 = nc.dram_tensor("xT0", [D, E], f32, kind="ExternalInput").ap()
    maskrep = nc.dram_tensor("maskrep", [HID, E], f32, kind="ExternalInput").ap()
    y0qD = nc.dram_tensor("y0q", [128, CH * YQ], fp8, kind="ExternalInput").ap()
    # packed weight blobs (single DMA each):
    # wblob bf16 (128, L*200): per layer i, cols [200i,200i+100) rows 0:NB = Ww[i],
    #   cols [200i+100,200i+200) rows 0:HID = Wh[i]
    # fblob f32 (128, 280): col i rows 0:HID = bias[i]; col 5+i = bh[i];
    #   cols [10+30(i-1),...) rows 0:D = wbx[i] (i>=1); row 0 cols 130..280 = c_all
    wblobD = nc.dram_tensor("wblob", [128, L * 200], bf16, kind="ExternalInput").ap()
    fblobD = nc.dram_tensor("fblob", [128, 408], f32, kind="ExternalInput").ap()
    graphD = nc.dram_tensor("graph", [HID, 1], f32, kind="ExternalOutput").ap()

    with tile.TileContext(nc) as tc:
        with (
            tc.tile_pool(name="singles", bufs=1) as singles,
            tc.tile_pool(name="resp", bufs=1) as resp,
            tc.tile_pool(name="ypool", bufs=2) as ypool,
            tc.tile_pool(name="hpool", bufs=2) as hpool,
            tc.tile_pool(name="zpool", bufs=3) as zpool,
            tc.tile_pool(name="tpool", bufs=2) as tpool,
            tc.tile_pool(name="zcpool", bufs=3) as zcpool,
            tc.tile_pool(name="psZ", bufs=3, space=MemorySpace.PSUM) as psZ,
            tc.tile_pool(name="psT", bufs=2, space=MemorySpace.PSUM) as psT,
            tc.tile_pool(name="psH", bufs=1, space=MemorySpace.PSUM) as psH,
            tc.tile_pool(name="psG", bufs=1, space=MemorySpace.PSUM) as psG,
            tc.tile_pool(name="psY", bufs=1, space=MemorySpace.PSUM) as psY,
        ):
            # ---- y0 first (layer 0 needs it immediately), then the big adj load ----
            ones = singles.tile([1, 128], f32, tag="ones", name="ones")
            nc.vector.memset(ones[:, :], 1.0)

            y0_sb = ypool.tile([128, CH * YQ], fp8, tag="y", name="y_all0")
            nc.sync.dma_start(out=y0_sb[:, :], in_=y0qD)

            # ---- resident adjT (the big load; r-ordered so layer 0 chases it) ----
            res_tiles = []
            for r in range(R):
                rt = resp.tile([128, C2 * 2 * E2], fp8, tag=f"res{r}", name=f"res{r}")
                nc.sync.dma_start(
                    out=rt[:, :].rearrange("p (c t i) -> p c t i", c=C2, t=2),
                    in_=adjT[r].rearrange("c p t i -> p c t i"),
                )
                res_tiles.append(rt)
            res_views = [
                res_tiles[r][:, :].rearrange("p (c t i) -> p c t i", c=C2, t=2)
                for r in range(R)
            ]

            # ---- late DMAs (needed only from the layer-0 tail onwards) ----
            wblob = singles.tile([128, L * 200], bf16, tag="wblob", name="wblob")
            nc.sync.dma_start(out=wblob[:, :], in_=wblobD)
            fblob = singles.tile([128, 408], f32, tag="fblob", name="fblob")
            nc.sync.dma_start(out=fblob[:, :], in_=fblobD)
            ww_sb = [wblob[0:NB, 200 * i: 200 * i + HID] for i in range(L)]
            wh_sb = [wblob[0:HID, 200 * i + 100: 200 * i + 200] for i in range(L)]
            bias_sb = [fblob[0:HID, i:i + 1] for i in range(L)]
            bh_sb = [fblob[0:HID, 5 + i: 6 + i] for i in range(L)]
            wbx_sb = [None] + [
                fblob[0:D, 10 + 30 * (i - 1): 10 + 30 * i] for i in range(1, L)
            ]
            c_views = [fblob[0:1, 130 + i * RNB: 130 + (i + 1) * RNB] for i in range(L)]
            ident = fblob[0:128, 280:408]

            xT = singles.tile([D, EP], f32, tag="xT", name="xT")
            nc.sync.dma_start(out=xT[:, 0:E], in_=xT0)
            nc.vector.memset(xT[:, E:EP], 0.0)

            mask_sb = singles.tile([HID, E], f32, tag="mask", name="mask_sb")
            nc.sync.dma_start(out=mask_sb[:, :], in_=maskrep)

            # reciprocal of clamped mask sum (off critical path)
            rden = singles.tile([HID, 1], f32, tag="rden", name="rden")
            nc.vector.reduce_sum(rden[:, :], mask_sb[:, :], axis=AX.X)
            nc.vector.tensor_scalar_max(rden[:, :], rden[:, :], 1.0)
            nc.vector.reciprocal(rden[:, :], rden[:, :])

            gpart = singles.tile([HID, 4], f32, tag="gpart", name="gpart")

            # ---------------- layer bodies ----------------
            def psy_chunk(i, k, y_all):
                """y[:, k-chunk] = x[:, k-chunk] @ Wbx[i] + 1 (x) c[i]"""
                psy = psY.tile([128, RNB], f32, tag="y", name=f"psy{i}_{k}")
                nc.tensor.matmul(
                    psy[:, :], xT[:, k * 128:(k + 1) * 128], wbx_sb[i],
                    start=True, stop=False,
                )
                nc.tensor.matmul(
                    psy[:, :], ones[:, :], c_views[i],
                    start=False, stop=True,
                )
                nc.scalar.copy(
                    out=y_all[:, k * YQ: k * YQ + RNB], in_=psy[:, :]
                )

            def z_chunk(i, ic, y_view, z_sb):
                """Z rows [128ic, 128ic+iw) accumulated e-major (adj stationary),
                then transposed into z_sb[ic//4] (nb-major)."""
                iw = 128 if ic < CH - 1 else E2 - 128 * (CH - 1)
                pz = psZ.tile([128, 4], f32, tag="pz", name=f"pz{i}_{ic}")
                for c in range(C2):
                    for r in range(R):
                        nc.tensor.matmul(
                            pz[0:iw, 0:NB],
                            res_views[r][:, c, :, 128 * ic: 128 * ic + iw],
                            y_view[:, 2 * c: 2 * c + 2, 3 * r: 3 * r + 3],
                            start=(c == 0 and r == 0),
                            stop=(c == C2 - 1 and r == R - 1),
                            perf_mode=DR,
                        )
                zcb = zcpool.tile([128, 4], f32, tag="zcb", name=f"zcb{i}_{ic}")
                nc.scalar.copy(out=zcb[0:iw, 0:NB], in_=pz[0:iw, 0:NB])
                pst = psT.tile([NB, 128], f32, tag="zt", name=f"zt{i}_{ic}")
                nc.tensor.transpose(pst[0:NB, 0:iw], zcb[0:iw, 0:NB],
                                    ident[0:iw, 0:iw])
                off = 128 * (ic % 4)
                nc.vector.tensor_copy(z_sb[ic // 4][:, off:off + iw],
                                      pst[0:NB, 0:iw])

            def tail_psh(i, w, z_sb, psh_w):
                ws, wn = WINS[w]
                nc.tensor.matmul(
                    psh_w[:, 0:wn], ww_sb[i], z_sb[:, 0:wn],
                    start=True, stop=True,
                )

            def tail_relu(i, w, psh_w, h_sb):
                ws, wn = WINS[w]
                nc.scalar.activation(
                    h_sb[:, ws:ws + wn], psh_w[:, 0:wn], AF.Relu,
                    bias=bias_sb[i],
                )

            def tail_psg(i, w, h_sb, psg_w):
                ws, wn = WINS[w]
                nc.tensor.matmul(
                    psg_w[:, 0:wn], wh_sb[i], h_sb[:, ws:ws + wn],
                    start=True, stop=True,
                )

            def tail_x(i, w, h_sb, psg_w):
                """sigmoid on ACT, then x += g*(h-x) on DVE.
                Last layer: fold the masked partial sum instead of writing x."""
                ws, wn = WINS[w]
                nc.scalar.activation(
                    psg_w[:, 0:wn], psg_w[:, 0:wn], AF.Sigmoid,
                    bias=bh_sb[i],
                )
                t = tpool.tile([HID, 512], f32, tag="t", name=f"t{i}_{w}")
                nc.vector.tensor_sub(t[:, 0:wn], h_sb[:, ws:ws + wn], xT[:, ws:ws + wn])
                nc.vector.tensor_mul(t[:, 0:wn], t[:, 0:wn], psg_w[:, 0:wn])
                if i < L - 1:
                    nc.vector.tensor_add(xT[:, ws:ws + wn], xT[:, ws:ws + wn], t[:, 0:wn])
                else:
                    nc.vector.tensor_add(t[:, 0:wn], xT[:, ws:ws + wn], t[:, 0:wn])
                    nc.vector.tensor_mul(t[:, 0:wn], t[:, 0:wn], mask_sb[:, ws:ws + wn])
                    nc.vector.reduce_sum(gpart[:, w:w + 1], t[:, 0:wn], axis=AX.X)

            # ---- all layers: chunked Z + window tails, pipelined ----
            y_view0 = y0_sb[:, :].rearrange("p (k q) -> p k q", q=YQ)
            prev_y = y0_sb
            for i in range(L):
                y_view = prev_y[:, :].rearrange("p (k q) -> p k q", q=YQ)
                z_sb = [zpool.tile([NB, 512], bf16, tag="z", name=f"z{i}_{w}")
                        for w in range(3)]
                h_sb = hpool.tile([HID, E], bf16, tag="h", name=f"h{i}")
                ps_h = [psH.tile([HID, 512], f32, tag="h", name=f"psh{i}_{w}")
                        for w in range(3)]
                ps_g = [psG.tile([HID, 512], f32, tag="g", name=f"psg{i}_{w}")
                        for w in range(3)]
                if i < L - 1:
                    next_y = ypool.tile([128, CH * YQ], fp8, tag="y",
                                        name=f"y_all{i+1}")

                for ic in range(4):
                    z_chunk(i, ic, y_view, z_sb)
                tail_psh(i, 0, z_sb[0], ps_h[0])
                tail_relu(i, 0, ps_h[0], h_sb)
                for ic in range(4, 8):
                    z_chunk(i, ic, y_view, z_sb)
                tail_psg(i, 0, h_sb, ps_g[0])
                tail_x(i, 0, h_sb, ps_g[0])
                for ic in range(8, CH):
                    z_chunk(i, ic, y_view, z_sb)
                tail_psh(i, 1, z_sb[1], ps_h[1])
                tail_relu(i, 1, ps_h[1], h_sb)
                if i < L - 1:
                    for k in range(4):
                        psy_chunk(i + 1, k, next_y)
                tail_psg(i, 1, h_sb, ps_g[1])
                tail_x(i, 1, h_sb, ps_g[1])
                tail_psh(i, 2, z_sb[2], ps_h[2])
                tail_relu(i, 2, ps_h[2], h_sb)
                if i < L - 1:
                    for k in range(4, 8):
                        psy_chunk(i + 1, k, next_y)
                tail_psg(i, 2, h_sb, ps_g[2])
                tail_x(i, 2, h_sb, ps_g[2])
                if i < L - 1:
                    for k in range(8, CH):
                        psy_chunk(i + 1, k, next_y)
                    prev_y = next_y

            # ---- masked mean epilogue (partials already in gpart) ----
            gsum = singles.tile([HID, 1], f32, tag="gsum", name="gsum")
            nc.vector.reduce_sum(gsum[:, :], gpart[:, 0:3], axis=AX.X)
            nc.vector.tensor_mul(gsum[:, :], gsum[:, :], rden[:, :])
            nc.sync.dma_start(out=graphD, in_=gsum[:, :])

    nc.compile()
    return nc


def get_nc():
    if "nc" not in _NC_CACHE:
        _NC_CACHE["nc"] = _build_nc()
    return _NC_CACHE["nc"]


def make_in_maps(adj, mask_ids, ent_emb, rel_emb, Wb, Ww, bias, Wh, bh):
    adj = np.asarray(adj, dtype=np.float32)
    pad = np.zeros((B, R, EP, E2), dtype=FP8_NP)
    pad[:, :, :E, :E] = adj.transpose(0, 1, 3, 2).astype(FP8_NP)
    # [b, r, c, p, t, i] = adj[b, r, i, j = c*256 + t*128 + p]
    adjT = np.ascontiguousarray(
        pad.reshape(B, R, C2, 2, 128, E2).transpose(0, 1, 2, 4, 3, 5)
    )
    ent = np.asarray(ent_emb, np.float32)
    entT = np.ascontiguousarray(ent.T)
    relf = np.asarray(rel_emb, np.float32)
    Wb5 = np.asarray(Wb, np.float32).reshape(L, R, 2, D, NB)
    wbx = np.ascontiguousarray(Wb5[:, :, 0].transpose(0, 2, 1, 3).reshape(L, D, RNB))
    # c[i, 3r+nb] = rel_r @ Wb_rel[i, r]
    c_all = np.einsum("rd,irdn->irn", relf, Wb5[:, :, 1]).reshape(L, RNB)
    # layer-0 y (x0 == ent_emb): (EP, RNB), pad rows get the c constant
    y0 = np.broadcast_to(c_all[0][None, :], (EP, RNB)).copy()
    y0[:E] += ent @ wbx[0]
    y0q = np.zeros((128, CH * YQ), dtype=FP8_NP)
    y0q_view = y0q.reshape(128, CH, YQ)
    y0_k = y0.reshape(CH, 128, RNB).transpose(1, 0, 2)   # (p, k, RNB)
    y0q_view[:, :, :RNB] = y0_k.astype(FP8_NP)
    maskf = np.asarray(mask_ids).astype(np.float32)
    wwb = np.asarray(Ww, np.float32).astype(ml_dtypes.bfloat16)
    whb = np.asarray(Wh, np.float32).astype(ml_dtypes.bfloat16)
    wblob = np.zeros((128, L * 200), dtype=ml_dtypes.bfloat16)
    fblob = np.zeros((128, 408), dtype=np.float32)
    fblob[:, 280:408] = np.eye(128, dtype=np.float32)
    biasf = np.asarray(bias, np.float32)
    bhf = np.asarray(bh, np.float32)
    for i in range(L):
        wblob[0:NB, 200 * i: 200 * i + HID] = wwb[i]
        wblob[0:HID, 200 * i + 100: 200 * i + 200] = whb[i]
        fblob[0:HID, i] = biasf[i]
        fblob[0:HID, 5 + i] = bhf[i]
        if i >= 1:
            fblob[0:D, 10 + 30 * (i - 1): 10 + 30 * i] = wbx[i]
    fblob[0, 130: 130 + L * RNB] = c_all.reshape(-1)
    common = dict(
        xT0=entT,
        y0q=y0q,
        wblob=wblob,
        fblob=fblob,
    )
    in_maps = []
    for c in range(8):
        b = c // 2
        m = dict(common)
        m["adjT"] = np.ascontiguousarray(adjT[b])
        m["maskrep"] = np.ascontiguousarray(
            np.broadcast_to(maskf[b][None, :], (HID, E))
        )
        in_maps.append(m)
    return in_maps


def run(inputs, trace=False):
    nc = get_nc()
    in_maps = make_in_maps(**{k: np.asarray(v) for k, v in inputs.items()})
    res = bass_utils.run_bass_kernel_spmd(
        nc, in_maps, core_ids=list(range(8)), trace=trace
    )
    out = np.stack(
        [np.asarray(res.results[2 * b]["graph"]).reshape(HID) for b in range(B)]
    ).astype(np.float32)
    return out, res


def kernel(**inputs):
    out, _ = run(inputs, trace=False)
    return out
